# revision 1
# baseline (speedup 1.0000x reference)
"""Trainium2 Bass kernel for nn_CognitiveProcessor.

Reference computation (per token, E=512, O=64):
  ph0   = tanh(x @ W1 + b1) @ W2 + b2                  [B,S,O]
  10 Euler steps: ph += DT*(omega + K*mean(sin(ph))*cos(ph))
  conc  = relu(ph @ W3 + b3) @ W4 + b4                 [B,S,E]
  out   = concat([conc, noise*fm, noise*fm, noise*fm], -1)  [B,S,E,4]
  with fm = sin(alpha*arange(E))

Sharding: pure data parallel over batch (B=8 -> 1 batch per core).
"""

import numpy as np

import concourse.bass as bass
import concourse.tile as tile
from concourse import mybir
from concourse.tile import add_dep_helper
from concourse.bass_utils import run_bass_kernel_spmd
from concourse.masks import make_identity

F32 = mybir.dt.float32
AF = mybir.ActivationFunctionType
OP = mybir.AluOpType

E = 512          # embed dim
O = 64           # oscillators
DT = 0.01
STEPS = 10
NCORES = 8
TOK = 4096       # tokens per core (one batch)
P = 128          # partitions / tokens per tile
NT = TOK // P    # 32 tiles per core
TPS = 8          # tiles per superblock
NSB = NT // TPS  # 4 superblocks
SBC = TPS * O    # phase columns per superblock = 512
HALF_PI = float(np.pi / 2)


def _bcast_ap(ap2d, n):
    """[P, G] -> [P, G, n] view with a step-0 innermost dim (free-dim bcast)."""
    return bass.AP(tensor=ap2d.tensor, offset=ap2d.offset, ap=[*ap2d.ap, [0, n]])


def _split_excess_waits(nc):
    """This toolchain's walrus allows at most 1 sync wait per ordinary
    instruction (2 on EventSemaphore). Hoist excess waits into same-engine
    EventSemaphore instructions inserted just before the offending
    instruction (waits are ANDed, so this is equivalent)."""
    import bass_rust as _br
    n = 0
    for f in nc.m.functions:
        for bb in f.blocks:
            old = bb.instructions
            new = []
            changed = False
            for inst in old:
                si = inst.sync_info
                waits = list(si.on_wait) if (si and si.on_wait) else []
                if len(waits) > 1:
                    changed = True
                    excess, waits = waits[:-1], waits[-1:]
                    while excess:
                        take, excess = excess[:2], excess[2:]
                        es = _br.InstEventSemaphore(name=f"wsplit_{n}")
                        n += 1
                        es.engine = inst.engine
                        es.sync_info = mybir.SyncInfo(on_wait=take, on_update=[])
                        new.append(es)
                    inst.sync_info = mybir.SyncInfo(
                        on_wait=waits,
                        on_update=list(si.on_update) if si.on_update else [])
                new.append(inst)
            if changed:
                bb.instructions = new
    return n


def _build(cdt, has_b2, has_b3, has_b4):
    nc = bass.Bass("TRN2", target_bir_lowering=False, debug=False,
                   enable_asserts=False)
    X = nc.dram_tensor("x", [TOK, E], F32, kind="ExternalInput")
    NZ = nc.dram_tensor("noise", [TOK, E], F32, kind="ExternalInput")
    W1 = nc.dram_tensor("w1", [P, 4, O], F32, kind="ExternalInput")
    W2 = nc.dram_tensor("w2", [O, O], F32, kind="ExternalInput")
    W3 = nc.dram_tensor("w3", [O, E], F32, kind="ExternalInput")
    W4 = nc.dram_tensor("w4", [P, 4, E], F32, kind="ExternalInput")
    B1 = nc.dram_tensor("b1", [O, 1], F32, kind="ExternalInput")
    OMG = nc.dram_tensor("omgrow", [1, SBC], F32, kind="ExternalInput")
    FM = nc.dram_tensor("fm", [E], F32, kind="ExternalInput")
    B2R = nc.dram_tensor("b2row", [1, SBC], F32, kind="ExternalInput")
    B3R = nc.dram_tensor("b3row", [1, E], F32, kind="ExternalInput")
    B4R = nc.dram_tensor("b4row", [1, E], F32, kind="ExternalInput")
    OUT = nc.dram_tensor("out", [TOK, 4 * E], F32, kind="ExternalOutput")

    last_dmas = []      # tail-ladder candidates (walrus sync-wait cap)
    last_eng = {}

    def D(inst):
        last_dmas.append(inst)
        return inst

    def EG(key, inst):
        last_eng[key] = inst
        return inst

    from contextlib import ExitStack
    with tile.TileContext(nc) as tc, ExitStack() as ctx:
        wp = ctx.enter_context(tc.tile_pool(name="w", bufs=1))
        xp = ctx.enter_context(tc.tile_pool(name="xp", bufs=4))
        xts = ctx.enter_context(tc.tile_pool(name="xts", bufs=3))
        p0s = ctx.enter_context(tc.tile_pool(name="p0s", bufs=3))
        sp = ctx.enter_context(tc.tile_pool(name="sp", bufs=2))
        mp = ctx.enter_context(tc.tile_pool(name="mp", bufs=2))
        pf = ctx.enter_context(tc.tile_pool(name="pf", bufs=2))
        pts = ctx.enter_context(tc.tile_pool(name="pts", bufs=3))
        h3p = ctx.enter_context(tc.tile_pool(name="h3p", bufs=3))
        nzp = ctx.enter_context(tc.tile_pool(name="nzp", bufs=4))
        op_ = ctx.enter_context(tc.tile_pool(name="op", bufs=3))
        aps = ctx.enter_context(tc.tile_pool(name="aps", bufs=3, space="PSUM"))
        php = ctx.enter_context(tc.tile_pool(name="php", bufs=2, space="PSUM"))
        cps = ctx.enter_context(tc.tile_pool(name="cps", bufs=3, space="PSUM"))

        # ---- one-time constants ----
        w1s = wp.tile([P, 4, O], F32)
        D(nc.sync.dma_start(out=w1s, in_=W1[:, :, :]))
        w2s = wp.tile([O, O], F32)
        D(nc.sync.dma_start(out=w2s, in_=W2[:, :]))
        w3s = wp.tile([O, E], F32)
        D(nc.sync.dma_start(out=w3s, in_=W3[:, :]))
        w4s = wp.tile([P, 4, E], F32)
        D(nc.sync.dma_start(out=w4s, in_=W4[:, :, :]))
        b1c = wp.tile([O, 1], F32)
        D(nc.sync.dma_start(out=b1c, in_=B1[:, :]))
        omgr = wp.tile([1, SBC], F32)
        D(nc.sync.dma_start(out=omgr, in_=OMG[:, :]))
        b2r = wp.tile([1, SBC], F32)
        D(nc.sync.dma_start(out=b2r, in_=B2R[:, :]))
        b3r = wp.tile([1, E], F32)
        D(nc.sync.dma_start(out=b3r, in_=B3R[:, :]))
        b4r = wp.tile([1, E], F32)
        D(nc.sync.dma_start(out=b4r, in_=B4R[:, :]))
        fmb = wp.tile([P, E], F32)
        fm_bcast = bass.AP(tensor=FM.ap().tensor, offset=0, ap=[[0, P], [1, E]])
        D(nc.gpsimd.dma_start(out=fmb, in_=fm_bcast))
        ident = wp.tile([P, P], F32)
        make_identity(nc, ident)
        ones = wp.tile([1, P], F32)
        EG("pool", nc.gpsimd.memset(ones, 1.0))
        halfpi = wp.tile([P, 1], F32)
        EG("dve", nc.vector.memset(halfpi, HALF_PI))

        for sb in range(NSB):
            ph = php.tile([P, SBC], F32)  # PSUM-resident phases for this sb

            # ---------- phase A: MLP1 ----------
            for g in range(TPS):
                t = sb * TPS + g
                xt = xp.tile([P, E], F32)
                D(nc.sync.dma_start(out=xt, in_=X[t * P:(t + 1) * P, :]))
                xT_ps = aps.tile([P, E], F32, tag="aps")
                for c in range(4):
                    EG("pe", nc.tensor.transpose(
                        xT_ps[:, c * P:(c + 1) * P], xt[:, c * P:(c + 1) * P],
                        ident))
                xTs = xts.tile([P, E], F32)
                EG("dve", nc.vector.tensor_copy(out=xTs, in_=xT_ps))
                p0 = aps.tile([O, P], F32, tag="aps")
                for c in range(4):
                    EG("pe", nc.tensor.matmul(
                        p0, w1s[:, c, :], xTs[:, c * P:(c + 1) * P],
                        start=(c == 0), stop=(c == 3)))
                p0t = p0s.tile([O, P], F32)
                EG("act", nc.scalar.activation(
                    out=p0t, in_=p0, func=AF.Tanh, bias=b1c, scale=1.0))
                # start only on g==0: start=True clears has_written for the
                # WHOLE bank, which would let the later Kuramoto accumulates
                # overwrite groups written before the last start.
                EG("pe", nc.tensor.matmul(
                    ph[:, g * O:(g + 1) * O], p0t, w2s,
                    start=(g == 0), stop=not has_b2,
                    skip_group_check=True))
                if has_b2:
                    EG("pe", nc.tensor.matmul(
                        ph[:, g * O:(g + 1) * O], ones,
                        b2r[:, g * O:(g + 1) * O],
                        start=False, stop=True, skip_group_check=True))

            # ---------- phase B: Kuramoto (batched over the superblock) ----
            for step in range(STEPS):
                s = sp.tile([P, SBC], F32, tag="s")
                EG("act", nc.scalar.activation(
                    out=s, in_=ph, func=AF.Sin, bias=0.0, scale=1.0))
                cs = sp.tile([P, SBC], F32, tag="c")
                EG("act", nc.scalar.activation(
                    out=cs, in_=ph, func=AF.Sin, bias=halfpi, scale=1.0))
                # omega add only needs sin/cos to have read ph — issue it
                # early so it overlaps the DVE reduce/STT on the chain.
                EG("pe", nc.tensor.matmul(
                    ph, ones, omgr, start=False, stop=True,
                    skip_group_check=True))
                msum = mp.tile([P, TPS], F32)
                s3 = s[:].rearrange("p (g o) -> p g o", o=O)
                EG("dve", nc.vector.tensor_reduce(
                    out=msum, in_=s3, axis=mybir.AxisListType.X, op=OP.add))
                u = sp.tile([P, SBC], F32, tag="u")
                u3 = u[:].rearrange("p (g o) -> p g o", o=O)
                c3 = cs[:].rearrange("p (g o) -> p g o", o=O)
                EG("dve", nc.vector.scalar_tensor_tensor(
                    out=u3, in0=_bcast_ap(msum[:], O), scalar=cdt, in1=c3,
                    op0=OP.mult, op1=OP.mult))
                EG("pe", nc.tensor.matmul(
                    ph, ident, u, start=False, stop=True,
                    skip_group_check=True))

            phf = pf.tile([P, SBC], F32)
            EG("act", nc.scalar.copy(out=phf, in_=ph))

            # ---------- phase C: MLP2 + quaternion assembly ----------
            for g in range(TPS):
                t = sb * TPS + g
                phT_ps = cps.tile([O, P], F32, tag="cps")
                EG("pe", nc.tensor.transpose(
                    phT_ps, phf[:, g * O:(g + 1) * O], ident))
                phTs = pts.tile([O, P], F32)
                EG("dve", nc.vector.tensor_copy(out=phTs, in_=phT_ps))
                h3 = cps.tile([P, E], F32, tag="cps")
                for c in range(4):
                    EG("pe", nc.tensor.matmul(
                        h3[:, c * P:(c + 1) * P], w3s[:, c * P:(c + 1) * P],
                        phTs, start=True, stop=not has_b3))
                    if has_b3:
                        EG("pe", nc.tensor.matmul(
                            h3[:, c * P:(c + 1) * P], b3r[:, c * P:(c + 1) * P],
                            ones, start=False, stop=True,
                            skip_group_check=True))
                h3s = h3p.tile([P, E], F32)
                EG("act", nc.scalar.activation(
                    out=h3s, in_=h3, func=AF.Relu, bias=0.0, scale=1.0))
                o4 = cps.tile([P, E], F32, tag="cps")
                for c in range(4):
                    EG("pe", nc.tensor.matmul(
                        o4, h3s[:, c * P:(c + 1) * P], w4s[:, c, :],
                        start=(c == 0), stop=(c == 3 and not has_b4)))
                if has_b4:
                    EG("pe", nc.tensor.matmul(
                        o4, ones, b4r, start=False, stop=True,
                        skip_group_check=True))

                nzt = nzp.tile([P, E], F32)
                D(nc.sync.dma_start(out=nzt, in_=NZ[t * P:(t + 1) * P, :]))
                ot = op_.tile([P, 4 * E], F32)
                v = ot[:].rearrange("p (e k) -> p e k", k=4)
                EG("act", nc.scalar.copy(out=v[:, :, 0], in_=o4))
                EG("dve", nc.vector.tensor_mul(
                    out=v[:, :, 1], in0=nzt, in1=fmb))
                EG("act", nc.scalar.copy(out=v[:, :, 2], in_=v[:, :, 1]))
                EG("dve", nc.vector.tensor_copy(out=v[:, :, 3], in_=v[:, :, 1]))
                D(nc.sync.dma_start(out=OUT[t * P:(t + 1) * P, :], in_=ot))

        # tail ladder: spread end-of-kernel sem waits across SP nops so the
        # final TileContext drain never needs >2 sync waits (walrus cap).
        tail = list(last_eng.values()) + last_dmas[-12:]
        for inst in tail:
            nop = nc.sync.nop()
            add_dep_helper(nop.ins, inst.ins, True, "tail ladder")

    _split_excess_waits(nc)
    return nc


_CACHE = {}


def kernel(x, noise, W1, b1, W2, b2, W3, b3, W4, b4, omega, K, alpha):
    x = np.asarray(x, dtype=np.float32)
    noise = np.asarray(noise, dtype=np.float32)
    W1 = np.asarray(W1, dtype=np.float32)
    W2 = np.asarray(W2, dtype=np.float32)
    W3 = np.asarray(W3, dtype=np.float32)
    W4 = np.asarray(W4, dtype=np.float32)
    b1 = np.asarray(b1, dtype=np.float32)
    b2 = np.asarray(b2, dtype=np.float32)
    b3 = np.asarray(b3, dtype=np.float32)
    b4 = np.asarray(b4, dtype=np.float32)
    omega = np.asarray(omega, dtype=np.float32)
    Kf = float(np.asarray(K))
    alphaf = float(np.asarray(alpha))

    B, S, Ein = x.shape
    assert (B, S, Ein) == (NCORES, TOK, E)

    cdt = Kf * DT / O
    has_b2 = bool(np.any(b2))
    has_b3 = bool(np.any(b3))
    has_b4 = bool(np.any(b4))
    key = (cdt, has_b2, has_b3, has_b4)
    if key not in _CACHE:
        _CACHE[key] = _build(*key)
    nc = _CACHE[key]

    # host-side prep of tiny params
    w1s = np.ascontiguousarray(W1.reshape(4, P, O).transpose(1, 0, 2))
    w4s = np.ascontiguousarray(W4.reshape(4, P, E).transpose(1, 0, 2))
    b1c = np.ascontiguousarray(b1.reshape(O, 1))
    omgrow = np.ascontiguousarray(np.tile(DT * omega, TPS).reshape(1, SBC))
    fm = np.sin(alphaf * np.arange(E, dtype=np.float32)).astype(np.float32)
    b2row = np.ascontiguousarray(np.tile(b2, TPS).reshape(1, SBC))
    b3row = np.ascontiguousarray(b3.reshape(1, E))
    b4row = np.ascontiguousarray(b4.reshape(1, E))

    in_maps = []
    for i in range(NCORES):
        in_maps.append({
            "x": np.ascontiguousarray(x[i]),
            "noise": np.ascontiguousarray(noise[i]),
            "w1": w1s, "w2": W2, "w3": W3, "w4": w4s,
            "b1": b1c, "omgrow": omgrow, "fm": fm,
            "b2row": b2row, "b3row": b3row, "b4row": b4row,
        })

    res = run_bass_kernel_spmd(nc, in_maps, core_ids=list(range(NCORES)))
    out = np.empty((B, S, E, 4), dtype=np.float32)
    for i in range(NCORES):
        out[i] = res.results[i]["out"].reshape(S, E, 4)
    return out



# revision 6
# speedup vs baseline: 1.6746x; 1.6746x over previous
"""Trainium2 Bass kernel for nn_CognitiveProcessor.

Reference computation (per token, E=512, O=64):
  ph0   = tanh(x @ W1 + b1) @ W2 + b2                  [B,S,O]
  10 Euler steps: ph += DT*(omega + K*mean(sin(ph))*cos(ph))
  conc  = relu(ph @ W3 + b3) @ W4 + b4                 [B,S,E]
  out   = concat([conc, noise*fm, noise*fm, noise*fm], -1)  [B,S,E,4]
  with fm = sin(alpha*arange(E))

Sharding: pure data parallel over batch (B=8 -> 1 batch per core).

v2 design notes (perf):
  - All matmul operands fp16 (1 cyc/row on PE vs 4 for fp32); PSUM
    accumulation stays fp32, Kuramoto phase state stays fp32 in PSUM.
  - x is transposed on host to [E, TOK] fp16 so MLP1 needs no on-chip
    transpose; noise shipped as fp16 (halves input DMA traffic).
  - Emission is software-pipelined: A(sb0,sb1) -> K(sb0,sb1 interleaved,
    with A(sb2,sb3) stuffed into the gaps) -> K(sb2,sb3) with C-tiles of
    the first half interleaved -> C of the second half.  Pair-wise
    Kuramoto matches the Act-engine rate (sin+cos ~1.1us/sb/step) to the
    per-step dependency chain (~2.2us), so output stores start ~30us in.
  - Elementwise work split: Act=sin/cos/tanh/relu/phf, DVE=reduce/stt/
    v1/v3, Pool=v0/v2 copies.
"""

import numpy as np

import concourse.bass as bass
import concourse.tile as tile
from concourse import mybir
from concourse.tile import add_dep_helper
from concourse.bass_utils import run_bass_kernel_spmd
from concourse.masks import make_identity

F32 = mybir.dt.float32
F16 = mybir.dt.float16
AF = mybir.ActivationFunctionType
OP = mybir.AluOpType

E = 512          # embed dim
O = 64           # oscillators
DT = 0.01
STEPS = 10
NCORES = 8
TOK = 4096       # tokens per core (one batch)
P = 128          # partitions / tokens per tile
NT = TOK // P    # 32 tiles per core
TPS = 8          # tiles per superblock
NSB = NT // TPS  # 4 superblocks
SBC = TPS * O    # phase columns per superblock = 512
TOKSB = TPS * P  # tokens per superblock = 1024
HALF_PI = float(np.pi / 2)


def _bcast_ap(ap2d, n):
    """[P, G] -> [P, G, n] view with a step-0 innermost dim (free-dim bcast)."""
    return bass.AP(tensor=ap2d.tensor, offset=ap2d.offset, ap=[*ap2d.ap, [0, n]])


def _split_excess_waits(nc):
    """This toolchain's walrus allows at most 1 sync wait per ordinary
    instruction (2 on EventSemaphore). Hoist excess waits into same-engine
    EventSemaphore instructions inserted just before the offending
    instruction (waits are ANDed, so this is equivalent)."""
    import bass_rust as _br
    n = 0
    for f in nc.m.functions:
        for bb in f.blocks:
            old = bb.instructions
            new = []
            changed = False
            for inst in old:
                si = inst.sync_info
                waits = list(si.on_wait) if (si and si.on_wait) else []
                if len(waits) > 1:
                    changed = True
                    excess, waits = waits[:-1], waits[-1:]
                    while excess:
                        take, excess = excess[:2], excess[2:]
                        es = _br.InstEventSemaphore(name=f"wsplit_{n}")
                        n += 1
                        es.engine = inst.engine
                        es.sync_info = mybir.SyncInfo(on_wait=take, on_update=[])
                        new.append(es)
                    inst.sync_info = mybir.SyncInfo(
                        on_wait=waits,
                        on_update=list(si.on_update) if si.on_update else [])
                new.append(inst)
            if changed:
                bb.instructions = new
    return n


def _build(cdt, has_b2, has_b3, has_b4):
    nc = bass.Bass("TRN2", target_bir_lowering=False, debug=False,
                   enable_asserts=False)
    XT = nc.dram_tensor("xt", [4, P, TOK], F16, kind="ExternalInput")
    NZ = nc.dram_tensor("noise", [TOK, E], F16, kind="ExternalInput")
    W1 = nc.dram_tensor("w1", [P, 4, O], F16, kind="ExternalInput")
    W2 = nc.dram_tensor("w2", [O, O], F16, kind="ExternalInput")
    W3 = nc.dram_tensor("w3", [O, E], F16, kind="ExternalInput")
    W4 = nc.dram_tensor("w4", [P, 4, E], F16, kind="ExternalInput")
    B1 = nc.dram_tensor("b1", [O, 1], F32, kind="ExternalInput")
    OMG = nc.dram_tensor("omgrow", [1, SBC], F16, kind="ExternalInput")
    FM = nc.dram_tensor("fm", [E], F16, kind="ExternalInput")
    B2R = nc.dram_tensor("b2row", [1, SBC], F16, kind="ExternalInput")
    B3R = nc.dram_tensor("b3row", [1, E], F16, kind="ExternalInput")
    B4R = nc.dram_tensor("b4row", [1, E], F16, kind="ExternalInput")
    OUT = nc.dram_tensor("out", [TOK, 4 * E], F32, kind="ExternalOutput")

    last_dmas = []      # tail-ladder candidates (walrus sync-wait cap)
    last_eng = {}

    def D(inst):
        last_dmas.append(inst)
        return inst

    def EG(key, inst):
        last_eng[key] = inst
        return inst

    from contextlib import ExitStack
    with tile.TileContext(nc) as tc, ExitStack() as ctx:
        wp = ctx.enter_context(tc.tile_pool(name="w", bufs=1))
        xsl = ctx.enter_context(tc.tile_pool(name="xsl", bufs=NSB))
        nzsl = ctx.enter_context(tc.tile_pool(name="nzsl", bufs=NSB))
        p0s = ctx.enter_context(tc.tile_pool(name="p0s", bufs=3))
        sp = ctx.enter_context(tc.tile_pool(name="sp", bufs=4))
        mp = ctx.enter_context(tc.tile_pool(name="mp", bufs=4))
        pf = ctx.enter_context(tc.tile_pool(name="pf", bufs=NSB))
        pts = ctx.enter_context(tc.tile_pool(name="pts", bufs=3))
        h3p = ctx.enter_context(tc.tile_pool(name="h3p", bufs=3))
        op_ = ctx.enter_context(tc.tile_pool(name="op", bufs=6))
        wps = ctx.enter_context(tc.tile_pool(name="wps", bufs=3, space="PSUM"))
        php = ctx.enter_context(tc.tile_pool(name="php", bufs=NSB,
                                             space="PSUM"))

        # ---- one-time constants ----
        w1s = wp.tile([P, 4, O], F16)
        D(nc.sync.dma_start(out=w1s, in_=W1[:, :, :]))
        w2s = wp.tile([O, O], F16)
        D(nc.sync.dma_start(out=w2s, in_=W2[:, :]))
        w3s = wp.tile([O, E], F16)
        D(nc.sync.dma_start(out=w3s, in_=W3[:, :]))
        w4s = wp.tile([P, 4, E], F16)
        D(nc.sync.dma_start(out=w4s, in_=W4[:, :, :]))
        b1c = wp.tile([O, 1], F32)
        D(nc.sync.dma_start(out=b1c, in_=B1[:, :]))
        omgr = wp.tile([1, SBC], F16)
        D(nc.sync.dma_start(out=omgr, in_=OMG[:, :]))
        b2r = wp.tile([1, SBC], F16)
        D(nc.sync.dma_start(out=b2r, in_=B2R[:, :]))
        b3r = wp.tile([1, E], F16)
        D(nc.sync.dma_start(out=b3r, in_=B3R[:, :]))
        b4r = wp.tile([1, E], F16)
        D(nc.sync.dma_start(out=b4r, in_=B4R[:, :]))
        fmb = wp.tile([P, E], F16)
        fm_bcast = bass.AP(tensor=FM.ap().tensor, offset=0, ap=[[0, P], [1, E]])
        D(nc.gpsimd.dma_start(out=fmb, in_=fm_bcast))
        ident = wp.tile([P, P], F16)
        make_identity(nc, ident)
        ones = wp.tile([1, P], F16)
        EG("pool", nc.gpsimd.memset(ones, 1.0))
        halfpi = wp.tile([P, 1], F32)
        EG("dve", nc.vector.memset(halfpi, HALF_PI))

        # ---- bulk input loads (big slabs; x first halves first) ----
        xslabs = []
        for sb in range(NSB):
            t_ = xsl.tile([P, 4, TOKSB], F16)
            src = bass.AP(tensor=XT.ap().tensor, offset=sb * TOKSB,
                          ap=[[TOK, P], [P * TOK, 4], [1, TOKSB]])
            D(nc.sync.dma_start(out=t_, in_=src))
            xslabs.append(t_)
        nzslabs = []
        for sb in range(NSB):
            t_ = nzsl.tile([P, TPS, E], F16)
            src = bass.AP(tensor=NZ.ap().tensor, offset=sb * TOKSB * E,
                          ap=[[E, P], [P * E, TPS], [1, E]])
            D(nc.sync.dma_start(out=t_, in_=src))
            nzslabs.append(t_)

        phs = [None] * NSB

        def emitA_group(sb, g):
            """MLP1 for tile (sb, g): fills ph[sb][:, g*O:(g+1)*O]."""
            p0 = wps.tile([O, P], F32, tag="wps")
            for c in range(4):
                EG("pe", nc.tensor.matmul(
                    p0, w1s[:, c, :],
                    xslabs[sb][:, c, g * P:(g + 1) * P],
                    start=(c == 0), stop=(c == 3)))
            p0t = p0s.tile([O, P], F16)
            EG("act", nc.scalar.activation(
                out=p0t, in_=p0, func=AF.Tanh, bias=b1c, scale=1.0))
            # start only on g==0: start=True clears has_written for the
            # WHOLE bank, which would let the later Kuramoto accumulates
            # overwrite groups written before the last start.
            EG("pe", nc.tensor.matmul(
                phs[sb][:, g * O:(g + 1) * O], p0t, w2s,
                start=(g == 0), stop=not has_b2,
                skip_group_check=True))
            if has_b2:
                EG("pe", nc.tensor.matmul(
                    phs[sb][:, g * O:(g + 1) * O], ones,
                    b2r[:, g * O:(g + 1) * O],
                    start=False, stop=True, skip_group_check=True))

        def emitK_step(sb):
            """One Kuramoto Euler step for superblock sb."""
            ph = phs[sb]
            s = sp.tile([P, SBC], F16, tag="s")
            EG("act", nc.scalar.activation(
                out=s, in_=ph, func=AF.Sin, bias=0.0, scale=1.0))
            cs = sp.tile([P, SBC], F16, tag="c")
            EG("act", nc.scalar.activation(
                out=cs, in_=ph, func=AF.Sin, bias=halfpi, scale=1.0))
            # omega add only needs sin/cos to have read ph — issue it
            # early so it overlaps the DVE reduce/STT on the chain.
            EG("pe", nc.tensor.matmul(
                ph, ones, omgr, start=False, stop=True,
                skip_group_check=True))
            msum = mp.tile([P, TPS], F32)
            s3 = s[:].rearrange("p (g o) -> p g o", o=O)
            EG("dve", nc.vector.tensor_reduce(
                out=msum, in_=s3, axis=mybir.AxisListType.X, op=OP.add))
            u = sp.tile([P, SBC], F16, tag="u")
            u3 = u[:].rearrange("p (g o) -> p g o", o=O)
            c3 = cs[:].rearrange("p (g o) -> p g o", o=O)
            EG("dve", nc.vector.scalar_tensor_tensor(
                out=u3, in0=_bcast_ap(msum[:], O), scalar=cdt, in1=c3,
                op0=OP.mult, op1=OP.mult))
            EG("pe", nc.tensor.matmul(
                ph, ident, u, start=False, stop=True,
                skip_group_check=True))

        phfs = [None] * NSB

        def emitPhf(sb):
            phf = pf.tile([P, SBC], F16)
            EG("act", nc.scalar.copy(out=phf, in_=phs[sb]))
            phfs[sb] = phf

        def emitC_tile(sb, g):
            """MLP2 + quaternion assembly + store for tile (sb, g)."""
            t = sb * TPS + g
            phT_ps = wps.tile([O, P], F16, tag="wps")
            EG("pe", nc.tensor.transpose(
                phT_ps, phfs[sb][:, g * O:(g + 1) * O], ident))
            phTs = pts.tile([O, P], F16)
            EG("dve", nc.vector.tensor_copy(out=phTs, in_=phT_ps))
            h3 = wps.tile([P, E], F32, tag="wps")
            for c in range(4):
                EG("pe", nc.tensor.matmul(
                    h3[:, c * P:(c + 1) * P], w3s[:, c * P:(c + 1) * P],
                    phTs, start=True, stop=not has_b3))
                if has_b3:
                    EG("pe", nc.tensor.matmul(
                        h3[:, c * P:(c + 1) * P], b3r[:, c * P:(c + 1) * P],
                        ones, start=False, stop=True,
                        skip_group_check=True))
            h3s = h3p.tile([P, E], F16)
            EG("act", nc.scalar.activation(
                out=h3s, in_=h3, func=AF.Relu, bias=0.0, scale=1.0))
            o4 = wps.tile([P, E], F32, tag="wps")
            for c in range(4):
                EG("pe", nc.tensor.matmul(
                    o4, h3s[:, c * P:(c + 1) * P], w4s[:, c, :],
                    start=(c == 0), stop=(c == 3 and not has_b4)))
            if has_b4:
                EG("pe", nc.tensor.matmul(
                    o4, ones, b4r, start=False, stop=True,
                    skip_group_check=True))

            nzt = nzslabs[sb][:, g, :]
            ot = op_.tile([P, 4 * E], F32)
            v = ot[:].rearrange("p (e k) -> p e k", k=4)
            EG("dve", nc.vector.scalar_tensor_tensor(
                out=v[:, :, 1], in0=nzt, scalar=1.0, in1=fmb,
                op0=OP.mult, op1=OP.mult))
            EG("dve", nc.vector.tensor_copy(out=v[:, :, 0], in_=o4))
            EG("pool", nc.gpsimd.tensor_copy(out=v[:, :, 2], in_=v[:, :, 1]))
            EG("pool", nc.gpsimd.tensor_copy(out=v[:, :, 3], in_=v[:, :, 1]))
            D(nc.sync.dma_start(out=OUT[t * P:(t + 1) * P, :], in_=ot))

        # ---------------- pipelined emission schedule ----------------
        for sb in range(NSB):
            phs[sb] = php.tile([P, SBC], F32, name="ph", tag="ph")

        # Phase A for the first half.
        for sb in (0, 1):
            for g in range(TPS):
                emitA_group(sb, g)

        # K(sb0, sb1) with A(sb2, sb3) stuffed into the gaps.
        for step in range(STEPS):
            emitK_step(0)
            emitK_step(1)
            if step < TPS:
                emitA_group(2, step)
                emitA_group(3, step)

        emitPhf(0)
        emitPhf(1)

        # K(sb2, sb3) with C tiles of the first half interleaved.
        for step in range(STEPS):
            emitK_step(2)
            emitK_step(3)
            if step < TPS:
                emitC_tile(0, step)
                if step % 4 == 3:
                    emitC_tile(1, step // 4 * 2)
                    emitC_tile(1, step // 4 * 2 + 1)

        emitPhf(2)
        emitPhf(3)
        for g in range(4, TPS):
            emitC_tile(1, g)
        for sb in (2, 3):
            for g in range(TPS):
                emitC_tile(sb, g)

        # tail ladder: spread end-of-kernel sem waits across SP nops so the
        # final TileContext drain never needs >2 sync waits (walrus cap).
        tail = list(last_eng.values()) + last_dmas[-12:]
        for inst in tail:
            nop = nc.sync.nop()
            add_dep_helper(nop.ins, inst.ins, True, "tail ladder")

    _split_excess_waits(nc)
    return nc


_CACHE = {}


def kernel(x, noise, W1, b1, W2, b2, W3, b3, W4, b4, omega, K, alpha):
    x = np.asarray(x, dtype=np.float32)
    noise = np.asarray(noise, dtype=np.float32)
    W1 = np.asarray(W1, dtype=np.float32)
    W2 = np.asarray(W2, dtype=np.float32)
    W3 = np.asarray(W3, dtype=np.float32)
    W4 = np.asarray(W4, dtype=np.float32)
    b1 = np.asarray(b1, dtype=np.float32)
    b2 = np.asarray(b2, dtype=np.float32)
    b3 = np.asarray(b3, dtype=np.float32)
    b4 = np.asarray(b4, dtype=np.float32)
    omega = np.asarray(omega, dtype=np.float32)
    Kf = float(np.asarray(K))
    alphaf = float(np.asarray(alpha))

    B, S, Ein = x.shape
    assert (B, S, Ein) == (NCORES, TOK, E)

    cdt = Kf * DT / O
    has_b2 = bool(np.any(b2))
    has_b3 = bool(np.any(b3))
    has_b4 = bool(np.any(b4))
    key = (cdt, has_b2, has_b3, has_b4)
    if key not in _CACHE:
        _CACHE[key] = _build(*key)
    nc = _CACHE[key]

    # host-side layout/dtype prep
    w1s = np.ascontiguousarray(
        W1.reshape(4, P, O).transpose(1, 0, 2)).astype(np.float16)
    w4s = np.ascontiguousarray(
        W4.reshape(4, P, E).transpose(1, 0, 2)).astype(np.float16)
    w2s = W2.astype(np.float16)
    w3s = W3.astype(np.float16)
    b1c = np.ascontiguousarray(b1.reshape(O, 1))
    omgrow = np.ascontiguousarray(
        np.tile(DT * omega, TPS).reshape(1, SBC)).astype(np.float16)
    fm = np.sin(alphaf * np.arange(E, dtype=np.float32)).astype(np.float16)
    b2row = np.ascontiguousarray(
        np.tile(b2, TPS).reshape(1, SBC)).astype(np.float16)
    b3row = np.ascontiguousarray(b3.reshape(1, E)).astype(np.float16)
    b4row = np.ascontiguousarray(b4.reshape(1, E)).astype(np.float16)

    in_maps = []
    for i in range(NCORES):
        xt = np.ascontiguousarray(x[i].T).astype(np.float16).reshape(4, P, TOK)
        in_maps.append({
            "xt": xt,
            "noise": noise[i].astype(np.float16),
            "w1": w1s, "w2": w2s, "w3": w3s, "w4": w4s,
            "b1": b1c, "omgrow": omgrow, "fm": fm,
            "b2row": b2row, "b3row": b3row, "b4row": b4row,
        })

    res = run_bass_kernel_spmd(nc, in_maps, core_ids=list(range(NCORES)))
    out = np.empty((B, S, E, 4), dtype=np.float32)
    for i in range(NCORES):
        out[i] = res.results[i]["out"].reshape(S, E, 4)
    return out


# revision 16
# speedup vs baseline: 1.8911x; 1.1292x over previous
"""Trainium2 Bass kernel for nn_CognitiveProcessor.

Reference computation (per token, E=512, O=64):
  ph0   = tanh(x @ W1 + b1) @ W2 + b2                  [B,S,O]
  10 Euler steps: ph += DT*(omega + K*mean(sin(ph))*cos(ph))
  conc  = relu(ph @ W3 + b3) @ W4 + b4                 [B,S,E]
  out   = concat([conc, noise*fm, noise*fm, noise*fm], -1)  [B,S,E,4]
  with fm = sin(alpha*arange(E))

Sharding: pure data parallel over batch (B=8 -> 1 batch per core).

v4 design notes (perf):
  - All matmul operands fp16 (1 cyc/row on PE); PSUM accumulation and
    the Kuramoto phase state stay fp32.
  - x transposed/cast to fp16 on host -> no on-chip input transposes;
    noise shipped fp16 (input DMA halved to 8MB/core).
  - Device output is PLANAR [TOK, 4, E]; the host permutes back.
    All SBUF writes contiguous; noise channels stored independently of
    the Kuramoto pipeline via a step-0 source AP that replicates one
    noise*fm tile into channels 1..3.
  - noise*fm runs on the otherwise-idle GPSIMD engine, 4 tiles per
    instruction; stores are batched 4 tiles per DMA to cut SP
    descriptor-generation time; conc stores issue from the Scalar
    queue, nz loads from the Vector queue (parallel DGE).
  - Kuramoto per step: Act sin/cos -> DVE group-reduce -> DVE mult ->
    DVE add of omega/cdt -> single PE accumulate through a cdt-scaled
    identity.  PE does one 512-row matmul per step per superblock.
  - PSUM: 4 ph banks + 2-deep {p0,phT,h3} ring + 2-deep {o4} ring.
  - Emission is software-pipelined: A(sb0,sb1) -> K(sb0,sb1) with
    A(sb2,sb3)+nm interleaved -> K(sb2,sb3) with C(first half)+nm
    interleaved -> C(second half).
"""

import numpy as np

import concourse.bass as bass
import concourse.tile as tile
from concourse import mybir
from concourse.tile import add_dep_helper
from concourse.bass_utils import run_bass_kernel_spmd
from concourse.masks import make_identity

F32 = mybir.dt.float32
F16 = mybir.dt.float16
AF = mybir.ActivationFunctionType
OP = mybir.AluOpType

E = 512          # embed dim
O = 64           # oscillators
DT = 0.01
STEPS = 10
NCORES = 8
TOK = 4096       # tokens per core (one batch)
P = 128          # partitions / tokens per tile
NT = TOK // P    # 32 tiles per core
TPS = 8          # tiles per superblock
NSB = NT // TPS  # 4 superblocks
SBC = TPS * O    # phase columns per superblock = 512
TOKSB = TPS * P  # tokens per superblock = 1024
HALF_PI = float(np.pi / 2)


def _bcast_ap(ap2d, n):
    """[P, G] -> [P, G, n] view with a step-0 innermost dim (free-dim bcast)."""
    return bass.AP(tensor=ap2d.tensor, offset=ap2d.offset, ap=[*ap2d.ap, [0, n]])


def _split_excess_waits(nc):
    """This toolchain's walrus allows at most 1 sync wait per ordinary
    instruction (2 on EventSemaphore). Hoist excess waits into same-engine
    EventSemaphore instructions inserted just before the offending
    instruction (waits are ANDed, so this is equivalent)."""
    import bass_rust as _br
    n = 0
    for f in nc.m.functions:
        for bb in f.blocks:
            old = bb.instructions
            new = []
            changed = False
            for inst in old:
                si = inst.sync_info
                waits = list(si.on_wait) if (si and si.on_wait) else []
                if len(waits) > 1:
                    changed = True
                    excess, waits = waits[:-1], waits[-1:]
                    while excess:
                        take, excess = excess[:2], excess[2:]
                        es = _br.InstEventSemaphore(name=f"wsplit_{n}")
                        n += 1
                        es.engine = inst.engine
                        es.sync_info = mybir.SyncInfo(on_wait=take, on_update=[])
                        new.append(es)
                    inst.sync_info = mybir.SyncInfo(
                        on_wait=waits,
                        on_update=list(si.on_update) if si.on_update else [])
                new.append(inst)
            if changed:
                bb.instructions = new
    return n


def _build(cdt, has_b2, has_b3, has_b4):
    nc = bass.Bass("TRN2", target_bir_lowering=False, debug=False,
                   enable_asserts=False)
    XT = nc.dram_tensor("xt", [4, P, TOK], F16, kind="ExternalInput")
    NZ = nc.dram_tensor("noise", [TOK, E], F16, kind="ExternalInput")
    W1 = nc.dram_tensor("w1", [P, 4, O], F16, kind="ExternalInput")
    W2 = nc.dram_tensor("w2", [O, O], F16, kind="ExternalInput")
    W3 = nc.dram_tensor("w3", [O, E], F16, kind="ExternalInput")
    W4 = nc.dram_tensor("w4", [P, 4, E], F16, kind="ExternalInput")
    B1 = nc.dram_tensor("b1", [O, 1], F32, kind="ExternalInput")
    # omega * DT / cdt, as a row; broadcast to all partitions on-chip
    OMG = nc.dram_tensor("omgrow", [1, SBC], F16, kind="ExternalInput")
    FM = nc.dram_tensor("fm", [E], F16, kind="ExternalInput")
    B2R = nc.dram_tensor("b2row", [1, SBC], F16, kind="ExternalInput")
    B3R = nc.dram_tensor("b3row", [1, E], F16, kind="ExternalInput")
    B4R = nc.dram_tensor("b4row", [1, E], F16, kind="ExternalInput")
    # planar: [token, channel, e]; host permutes to [token, e, channel]
    OUT = nc.dram_tensor("out", [TOK, 4, E], F32, kind="ExternalOutput")

    last_dmas = []      # tail-ladder candidates (walrus sync-wait cap)
    last_eng = {}

    def D(inst):
        last_dmas.append(inst)
        return inst

    def EG(key, inst):
        last_eng[key] = inst
        return inst

    from contextlib import ExitStack
    with tile.TileContext(nc) as tc, ExitStack() as ctx:
        wp = ctx.enter_context(tc.tile_pool(name="w", bufs=1))
        xsl = ctx.enter_context(tc.tile_pool(name="xsl", bufs=NSB))
        nzsl = ctx.enter_context(tc.tile_pool(name="nzsl", bufs=NSB))
        p0s = ctx.enter_context(tc.tile_pool(name="p0s", bufs=3))
        sp = ctx.enter_context(tc.tile_pool(name="sp", bufs=4))
        mp = ctx.enter_context(tc.tile_pool(name="mp", bufs=4))
        pf = ctx.enter_context(tc.tile_pool(name="pf", bufs=NSB))
        pts = ctx.enter_context(tc.tile_pool(name="pts", bufs=3))
        h3p = ctx.enter_context(tc.tile_pool(name="h3p", bufs=3))
        nmp = ctx.enter_context(tc.tile_pool(name="nmp", bufs=2))
        ocp = ctx.enter_context(tc.tile_pool(name="ocp", bufs=3))
        wps = ctx.enter_context(tc.tile_pool(name="wps", bufs=2, space="PSUM"))
        o4p = ctx.enter_context(tc.tile_pool(name="o4p", bufs=2, space="PSUM"))
        php = ctx.enter_context(tc.tile_pool(name="php", bufs=NSB,
                                             space="PSUM"))

        # ---- constants needed by phase A / K first ----
        w1s = wp.tile([P, 4, O], F16)
        D(nc.sync.dma_start(out=w1s, in_=W1[:, :, :]))
        w2s = wp.tile([O, O], F16)
        D(nc.sync.dma_start(out=w2s, in_=W2[:, :]))
        b1c = wp.tile([O, 1], F32)
        D(nc.sync.dma_start(out=b1c, in_=B1[:, :]))
        # omega/cdt broadcast to [P, SBC] (gpsimd DMA, off the SP queue)
        omgf = wp.tile([P, SBC], F16)
        omg_bcast = bass.AP(tensor=OMG.ap().tensor, offset=0,
                            ap=[[0, P], [1, SBC]])
        D(nc.gpsimd.dma_start(out=omgf, in_=omg_bcast))

        # ---- bulk input loads (x/consts on SP, nz on DVE queue) ----
        xslabs = [None] * NSB
        nzslabs = [None] * NSB

        def load_x(sb):
            t_ = xsl.tile([P, 4, TOKSB], F16, name="xslab")
            src = bass.AP(tensor=XT.ap().tensor, offset=sb * TOKSB,
                          ap=[[TOK, P], [P * TOK, 4], [1, TOKSB]])
            D(nc.sync.dma_start(out=t_, in_=src))
            xslabs[sb] = t_

        def load_nz(sb):
            t_ = nzsl.tile([P, TPS, E], F16, name="nzslab")
            src = bass.AP(tensor=NZ.ap().tensor, offset=sb * TOKSB * E,
                          ap=[[E, P], [P * E, TPS], [1, E]])
            D(nc.scalar.dma_start(out=t_, in_=src))
            nzslabs[sb] = t_

        load_x(0)
        load_x(1)
        load_nz(0)
        load_nz(1)

        # ---- remaining constants ----
        w3s = wp.tile([O, E], F16)
        D(nc.sync.dma_start(out=w3s, in_=W3[:, :]))
        w4s = wp.tile([P, 4, E], F16)
        D(nc.sync.dma_start(out=w4s, in_=W4[:, :, :]))
        b2r = wp.tile([1, SBC], F16)
        D(nc.sync.dma_start(out=b2r, in_=B2R[:, :]))
        b3r = wp.tile([1, E], F16)
        D(nc.sync.dma_start(out=b3r, in_=B3R[:, :]))
        b4r = wp.tile([1, E], F16)
        D(nc.sync.dma_start(out=b4r, in_=B4R[:, :]))
        fmb = wp.tile([P, E], F16)
        fm_bcast = bass.AP(tensor=FM.ap().tensor, offset=0, ap=[[0, P], [1, E]])
        D(nc.gpsimd.dma_start(out=fmb, in_=fm_bcast))
        ident = wp.tile([P, P], F16)
        make_identity(nc, ident)
        # identity scaled by cdt: folds the Kuramoto coupling constant into
        # the PSUM-accumulate matmul.
        identc = wp.tile([P, P], F16)
        nc.gpsimd.memset(identc, 0.0)
        nc.gpsimd.affine_select(
            out=identc, in_=identc, compare_op=OP.not_equal, fill=float(cdt),
            base=0, pattern=[[-1, P]], channel_multiplier=1)
        ones = wp.tile([1, P], F16)
        EG("pool", nc.gpsimd.memset(ones, 1.0))
        halfpi = wp.tile([P, 1], F32)
        EG("dve", nc.vector.memset(halfpi, HALF_PI))

        load_x(2)
        load_x(3)
        load_nz(2)
        load_nz(3)

        phs = [None] * NSB

        def emitA_group(sb, g):
            """MLP1 for tile (sb, g): fills ph[sb][:, g*O:(g+1)*O]."""
            p0 = wps.tile([O, P], F32, tag="wps", name="p0")
            for c in range(4):
                EG("pe", nc.tensor.matmul(
                    p0, w1s[:, c, :],
                    xslabs[sb][:, c, g * P:(g + 1) * P],
                    start=(c == 0), stop=(c == 3)))
            p0t = p0s.tile([O, P], F16)
            EG("act", nc.scalar.activation(
                out=p0t, in_=p0, func=AF.Tanh, bias=b1c, scale=1.0))
            # start only on g==0: start=True clears has_written for the
            # WHOLE bank, which would let the later Kuramoto accumulates
            # overwrite groups written before the last start.
            EG("pe", nc.tensor.matmul(
                phs[sb][:, g * O:(g + 1) * O], p0t, w2s,
                start=(g == 0), stop=not has_b2,
                skip_group_check=True))
            if has_b2:
                EG("pe", nc.tensor.matmul(
                    phs[sb][:, g * O:(g + 1) * O], ones,
                    b2r[:, g * O:(g + 1) * O],
                    start=False, stop=True, skip_group_check=True))

        def emitK_step(sb):
            """One Kuramoto Euler step for superblock sb."""
            ph = phs[sb]
            s = sp.tile([P, SBC], F16, tag="s", name="s")
            EG("act", nc.scalar.activation(
                out=s, in_=ph, func=AF.Sin, bias=0.0, scale=1.0))
            cs = sp.tile([P, SBC], F16, tag="c", name="cs")
            EG("act", nc.scalar.activation(
                out=cs, in_=ph, func=AF.Sin, bias=halfpi, scale=1.0))
            msum = mp.tile([P, TPS], F32, name="msum")
            s3 = s[:].rearrange("p (g o) -> p g o", o=O)
            EG("dve", nc.vector.tensor_reduce(
                out=msum, in_=s3, axis=mybir.AxisListType.X, op=OP.add))
            u = sp.tile([P, SBC], F16, tag="u", name="u")
            u3 = u[:].rearrange("p (g o) -> p g o", o=O)
            c3 = cs[:].rearrange("p (g o) -> p g o", o=O)
            EG("dve", nc.vector.tensor_mul(
                out=u3, in0=c3, in1=_bcast_ap(msum[:], O)))
            u2 = sp.tile([P, SBC], F16, tag="u2", name="u2")
            EG("dve", nc.vector.tensor_add(out=u2, in0=u, in1=omgf))
            # ph += cdt * u2  (cdt folded into the scaled identity)
            EG("pe", nc.tensor.matmul(
                ph, identc, u2, start=False, stop=True,
                skip_group_check=True))

        phfs = [None] * NSB

        def emitPhf(sb):
            phf = pf.tile([P, SBC], F16, name="phf")
            EG("act", nc.scalar.copy(out=phf, in_=phs[sb]))
            phfs[sb] = phf

        def emitNM8(sb):
            """noise*fm for all 8 tiles of superblock sb on GPSIMD; stored as
            3 channel-DMAs (each a 3D AP over 8 token tiles)."""
            t = sb * TPS
            nm8 = nmp.tile([P, TPS, E], F32, name="nm8")
            fm8 = bass.AP(tensor=fmb[:].tensor, offset=fmb[:].offset,
                          ap=[fmb[:].ap[0], [0, TPS], fmb[:].ap[1]])
            EG("pool", nc.gpsimd.tensor_mul(
                out=nm8, in0=nzslabs[sb][:, :, :], in1=fm8))
            for ch in range(1, 4):
                dst = bass.AP(tensor=OUT.ap().tensor,
                              offset=t * P * 4 * E + ch * E,
                              ap=[[4 * E, P], [P * 4 * E, TPS], [1, E]])
                D(nc.sync.dma_start(out=dst, in_=nm8[:]))

        oc4_state = [None, 0]   # current oc4 tile, fill count

        def emitC_tile(sb, g):
            """MLP2 for tile (sb, g); conc stores batched 4 tiles."""
            t = sb * TPS + g
            phT_ps = wps.tile([O, P], F16, tag="wps", name="phT_ps")
            EG("pe", nc.tensor.transpose(
                phT_ps, phfs[sb][:, g * O:(g + 1) * O], ident))
            phTs = pts.tile([O, P], F16)
            EG("dve", nc.vector.tensor_copy(out=phTs, in_=phT_ps))
            h3 = wps.tile([P, E], F32, tag="wps", name="h3")
            for c in range(4):
                EG("pe", nc.tensor.matmul(
                    h3[:, c * P:(c + 1) * P], w3s[:, c * P:(c + 1) * P],
                    phTs, start=True, stop=not has_b3))
                if has_b3:
                    EG("pe", nc.tensor.matmul(
                        h3[:, c * P:(c + 1) * P], b3r[:, c * P:(c + 1) * P],
                        ones, start=False, stop=True,
                        skip_group_check=True))
            h3s = h3p.tile([P, E], F16)
            EG("dve", nc.vector.tensor_scalar_max(
                out=h3s, in0=h3, scalar1=0.0))
            o4 = o4p.tile([P, E], F32, name="o4")
            for c in range(4):
                EG("pe", nc.tensor.matmul(
                    o4, h3s[:, c * P:(c + 1) * P], w4s[:, c, :],
                    start=(c == 0), stop=(c == 3 and not has_b4)))
            if has_b4:
                EG("pe", nc.tensor.matmul(
                    o4, ones, b4r, start=False, stop=True,
                    skip_group_check=True))
            if oc4_state[1] == 0:
                oc4_state[0] = ocp.tile([P, 4, E], F32, name="oc4")
            oc4, j = oc4_state
            EG("act", nc.scalar.copy(out=oc4[:, j, :], in_=o4))
            oc4_state[1] += 1
            if oc4_state[1] == 4:
                oc4_state[1] = 0
                dst = bass.AP(tensor=OUT.ap().tensor,
                              offset=(t - 3) * P * 4 * E,
                              ap=[[4 * E, P], [P * 4 * E, 4], [1, E]])
                D(nc.scalar.dma_start(out=dst, in_=oc4[:]))

        # ---------------- pipelined emission schedule ----------------
        for sb in range(NSB):
            phs[sb] = php.tile([P, SBC], F32, name="ph", tag="ph")

        # Phase A for the first half.
        for sb in (0, 1):
            for g in range(TPS):
                emitA_group(sb, g)

        # K(sb0, sb1) with A(sb2, sb3) and nm batches stuffed into the gaps.
        nm_list = list(range(NSB))
        for step in range(STEPS):
            emitK_step(0)
            emitK_step(1)
            if step < TPS:
                emitA_group(2, step)
                emitA_group(3, step)
            if step in (1, 5):
                emitNM8(nm_list.pop(0))

        emitPhf(0)
        emitPhf(1)

        # K(sb2, sb3) with C tiles of the first half and more nm interleaved.
        for step in range(STEPS):
            emitK_step(2)
            emitK_step(3)
            if step < TPS:
                sb01 = 0 if step < 4 else 1
                emitC_tile(sb01, 2 * (step % 4))
                emitC_tile(sb01, 2 * (step % 4) + 1)
            if step in (1, 5) and nm_list:
                emitNM8(nm_list.pop(0))

        emitPhf(2)
        emitPhf(3)
        for sb in (2, 3):
            for g in range(TPS):
                emitC_tile(sb, g)

        # tail ladder: spread end-of-kernel sem waits across SP nops so the
        # final TileContext drain never needs >2 sync waits (walrus cap).
        tail = list(last_eng.values()) + last_dmas[-12:]
        for inst in tail:
            nop = nc.sync.nop()
            add_dep_helper(nop.ins, inst.ins, True, "tail ladder")

    _split_excess_waits(nc)
    return nc


_CACHE = {}


def kernel(x, noise, W1, b1, W2, b2, W3, b3, W4, b4, omega, K, alpha):
    x = np.asarray(x, dtype=np.float32)
    noise = np.asarray(noise, dtype=np.float32)
    W1 = np.asarray(W1, dtype=np.float32)
    W2 = np.asarray(W2, dtype=np.float32)
    W3 = np.asarray(W3, dtype=np.float32)
    W4 = np.asarray(W4, dtype=np.float32)
    b1 = np.asarray(b1, dtype=np.float32)
    b2 = np.asarray(b2, dtype=np.float32)
    b3 = np.asarray(b3, dtype=np.float32)
    b4 = np.asarray(b4, dtype=np.float32)
    omega = np.asarray(omega, dtype=np.float32)
    Kf = float(np.asarray(K))
    alphaf = float(np.asarray(alpha))

    B, S, Ein = x.shape
    assert (B, S, Ein) == (NCORES, TOK, E)

    cdt = Kf * DT / O
    has_b2 = bool(np.any(b2))
    has_b3 = bool(np.any(b3))
    has_b4 = bool(np.any(b4))
    key = (cdt, has_b2, has_b3, has_b4)
    if key not in _CACHE:
        _CACHE[key] = _build(*key)
    nc = _CACHE[key]

    # host-side layout/dtype prep
    w1s = np.ascontiguousarray(
        W1.reshape(4, P, O).transpose(1, 0, 2)).astype(np.float16)
    w4s = np.ascontiguousarray(
        W4.reshape(4, P, E).transpose(1, 0, 2)).astype(np.float16)
    w2s = W2.astype(np.float16)
    w3s = W3.astype(np.float16)
    b1c = np.ascontiguousarray(b1.reshape(O, 1))
    # omega premultiplied by DT and divided by cdt (cdt reapplied by the
    # scaled-identity PSUM accumulate)
    omgrow = np.ascontiguousarray(
        np.tile(DT * omega / cdt, TPS).reshape(1, SBC)).astype(np.float16)
    fm = np.sin(alphaf * np.arange(E, dtype=np.float32)).astype(np.float16)
    b2row = np.ascontiguousarray(
        np.tile(b2, TPS).reshape(1, SBC)).astype(np.float16)
    b3row = np.ascontiguousarray(b3.reshape(1, E)).astype(np.float16)
    b4row = np.ascontiguousarray(b4.reshape(1, E)).astype(np.float16)

    in_maps = []
    for i in range(NCORES):
        xt = np.ascontiguousarray(x[i].T).astype(np.float16).reshape(4, P, TOK)
        in_maps.append({
            "xt": xt,
            "noise": noise[i].astype(np.float16),
            "w1": w1s, "w2": w2s, "w3": w3s, "w4": w4s,
            "b1": b1c, "omgrow": omgrow, "fm": fm,
            "b2row": b2row, "b3row": b3row, "b4row": b4row,
        })

    res = run_bass_kernel_spmd(nc, in_maps, core_ids=list(range(NCORES)))
    out = np.empty((B, S, E, 4), dtype=np.float32)
    for i in range(NCORES):
        # device layout is [token, channel, e] -> permute to [token, e, ch]
        out[i] = res.results[i]["out"].reshape(S, 4, E).transpose(0, 2, 1)
    return out
